# revision 32
# baseline (speedup 1.0000x reference)
"""Trainium2 Bass kernel for nn_MixedDiffEncoder (vq_codebook).

Math (see reference):
  loss[b] = mean_d( (xp2 @ w_proj + xd2 - images)^2 )
with the two heavy contractions against the (K, D*E2) codebook w_img2
restructured into dense matmuls:
  encode:  A_raw[b,k,e] = sum_d images[b,d] * w_img2[k,d,e]   (k-sharded)
           lat2[b,e]    = sum_k xp[b,k]*s[k,e]*(A_raw - c_raw)[b,k,e]
  decode:  xd2[b,d]     = sum_{k,e} xp2[b,k]*lat2[b,e]*s[k,e]*w_img2[k,d,e]
           (d-sharded, host-pretransposed to (k,e)-major layout)
s[k,e] = 1/(0.01 + std_d) is computed on-device from column sums (a ones
column folded into the encode stationary) and square-sums (scalar-engine
square + ones-matmul). Softmaxes drop per-row constants (invariant).

Sharding over 8 cores: encode k-parallel (64 k each), decode d-parallel
(384 d each). Cross-core collectives: AllGather of s shards, AllReduce of
lat2 partials, AllReduce of per-d-slice loss partials.
"""

import sys

for _p in ("/opt/trn_rl_repo", "/root/.axon_site/_ro/trn_rl_repo"):
    if _p not in sys.path:
        sys.path.append(_p)

import numpy as np

B, D, K, E2 = 64, 3072, 512, 64
NCORES = 8
KC = K // NCORES          # 64 k per core (encode shard)
DC = D // NCORES          # 384 d per core (decode shard)
G = 8                     # k-groups per core
J = KC // G               # 8 k per group
T = D // 128              # 24 contraction chunks
MSTAT = 64 + 1 + J        # 73 stationary columns (imagesT | ones | wprojT)
CC = (K * E2) // 128      # 256 decode contraction chunks
DECSL = 16                # decode chunks per DMA slice
WPQ = 6                   # wpT chunks per DMA slice
BETA = 5.0

_PROGRAM_CACHE = {}


def _build_program(debug=False):
    import concourse.bass as bass
    import concourse.bacc as bacc
    import concourse.mybir as mybir
    import concourse.tile as tile
    from contextlib import ExitStack

    f16 = mybir.dt.float16
    f32 = mybir.dt.float32
    bf16 = mybir.dt.bfloat16
    AF = mybir.ActivationFunctionType
    ALU = mybir.AluOpType
    AX = mybir.AxisListType
    RG = [list(range(NCORES))]

    nc = bacc.Bacc()

    # ---- I/O ----
    enc_w = nc.declare_dram_parameter("enc_w", [128, G * T * J * E2], f16, isOutput=False)
    enc_st = nc.declare_dram_parameter("enc_st", [128, G * T * MSTAT], f16, isOutput=False)
    imT = nc.declare_dram_parameter("imT", [128, T * 64], f16, isOutput=False)
    wpT = nc.declare_dram_parameter("wpT", [128, T * K], f16, isOutput=False)
    dec_w = nc.declare_dram_parameter("dec_w", [128, CC * DC], f16, isOutput=False)
    wp_dec = nc.declare_dram_parameter("wp_dec", [128, 4 * DC], f16, isOutput=False)
    wrec_t = nc.declare_dram_parameter("wrec_t", [128, 4 * E2], f32, isOutput=False)
    wrec_n = nc.declare_dram_parameter("wrec_n", [E2, K], f32, isOutput=False)
    x_dsl = nc.declare_dram_parameter("x_dsl", [B, DC], f32, isOutput=False)
    sel = nc.declare_dram_parameter("sel", [128, 4 * KC], f32, isOutput=False)
    ident = nc.declare_dram_parameter("ident", [128, 128], f32, isOutput=False)
    loss_o = nc.declare_dram_parameter("loss", [B], f32, isOutput=True)

    dbg = {}
    if debug:
        for name, shape in [
            ("dbg_xp", [B, K]), ("dbg_lat1", [B, E2]), ("dbg_xp2", [B, K]),
            ("dbg_s", [KC, E2]), ("dbg_lat2p", [B, E2]),
            ("dbg_lat2", [B, E2]), ("dbg_xd2", [B, DC]),
        ]:
            dbg[name] = nc.declare_dram_parameter(name, shape, f32, isOutput=True)
        dbg["dbg_xxt"] = nc.declare_dram_parameter("dbg_xxt", [128, CC * B], f16,
                                                   isOutput=True)

    # ---- internal DRAM (collective bounce buffers) ----
    # one AllGather carries both the s shard (first 4096) and the lat2
    # partial (last 4096) per rank
    SL = KC * E2              # 4096
    xp2_dr = nc.dram_tensor("xp2_dr", [K, B], f16)
    sl_in = nc.dram_tensor("sl_in", [1, 2 * SL], f32)
    sl_ag = nc.dram_tensor("sl_ag", [NCORES, 2 * SL], f32, addr_space="Shared")
    loss_in = nc.dram_tensor("loss_in", [B, 1], f32)
    loss_ag = nc.dram_tensor("loss_ag", [NCORES * B, 1], f32, addr_space="Shared")

    def pb(ap, n):
        """Broadcast a (1, ...) AP across n partitions (partition step 0)."""
        return bass.AP(tensor=ap.tensor, offset=ap.offset,
                       ap=[[0, n]] + [list(d) for d in ap.ap[1:]])

    def eb(ap, n):
        """Append an innermost free broadcast dim (step 0, count n)."""
        return bass.AP(tensor=ap.tensor, offset=ap.offset,
                       ap=[list(d) for d in ap.ap] + [[0, n]])

    with tile.TileContext(nc) as tc, ExitStack() as ctx:
        const = ctx.enter_context(tc.tile_pool(name="const", bufs=1))
        pers = ctx.enter_context(tc.tile_pool(name="pers", bufs=1))
        scr = ctx.enter_context(tc.tile_pool(name="scr", bufs=2))
        estp = ctx.enter_context(tc.tile_pool(name="estp", bufs=2))
        wptp = ctx.enter_context(tc.tile_pool(name="wptp", bufs=2))
        ewp = ctx.enter_context(tc.tile_pool(name="ewp", bufs=2))
        dwp = ctx.enter_context(tc.tile_pool(name="dwp", bufs=2))
        sqp = ctx.enter_context(tc.tile_pool(name="sqp", bufs=3))
        psm = ctx.enter_context(tc.tile_pool(name="psm", bufs=3, space="PSUM"))
        psa = ctx.enter_context(tc.tile_pool(name="psa", bufs=2, space="PSUM"))
        pss = ctx.enter_context(tc.tile_pool(name="pss", bufs=2, space="PSUM"))
        psx = ctx.enter_context(tc.tile_pool(name="psx", bufs=1, space="PSUM"))

        # ---- constant loads ----
        ident_sb = const.tile([128, 128], f32, name="ident_sb")
        nc.sync.dma_start(out=ident_sb, in_=ident[:, :])
        ones_bf = const.tile([128, 1], bf16, name="ones_bf")
        nc.vector.memset(ones_bf, 1.0)
        imT_sb = const.tile([128, T * 64], f16, name="imT_sb")
        nc.sync.dma_start(out=imT_sb, in_=imT[:, :])
        wrec_t_sb = const.tile([128, 4 * E2], f32, name="wrec_t_sb")
        nc.sync.dma_start(out=wrec_t_sb, in_=wrec_t[:, :])
        wrec_n_sb = const.tile([E2, K], f32, name="wrec_n_sb")
        nc.sync.dma_start(out=wrec_n_sb, in_=wrec_n[:, :])
        xdsl_sb = const.tile([B, DC], f32, name="xdsl_sb")
        nc.sync.dma_start(out=xdsl_sb, in_=x_dsl[:, :])
        sel_sb = const.tile([128, 4 * KC], f32, name="sel_sb")
        nc.sync.dma_start(out=sel_sb, in_=sel[:, :])
        wp_dec_sb = const.tile([128, 4 * DC], f16, name="wp_dec_sb")
        nc.sync.dma_start(out=wp_dec_sb, in_=wp_dec[:, :])

        # ---- phase 0: xh logits = (2b/D) images@w_proj.T - (b/D)||w_k||^2 ----
        ps_xh = psm.tile([128, 512], f32, name="pscr")
        ps_wp2 = psm.tile([128, 512], f32, name="pscr")
        for q in range(T // WPQ):
            wpq = wptp.tile([128, WPQ * K], f16, name="wpq")
            dmae = nc.sync if q % 2 == 0 else nc.scalar
            dmae.dma_start(out=wpq, in_=wpT[:, q * WPQ * K:(q + 1) * WPQ * K])
            for i in range(WPQ):
                t = q * WPQ + i
                nc.tensor.matmul(ps_xh[:64, :], lhsT=imT_sb[:, t * 64:(t + 1) * 64],
                                 rhs=wpq[:, i * K:(i + 1) * K],
                                 start=(t == 0), stop=(t == T - 1))
            for i in range(WPQ):
                t = q * WPQ + i
                sq = sqp.tile([128, 512], bf16, name="sq")
                nc.scalar.activation(out=sq, in_=wpq[:, i * K:(i + 1) * K], func=AF.Square)
                nc.tensor.matmul(ps_wp2[:1, :], lhsT=ones_bf, rhs=sq,
                                 start=(t == 0), stop=(t == T - 1))

        wp2r = pers.tile([1, K], f32, name="wp2r")
        nc.vector.tensor_scalar_mul(wp2r, ps_wp2[:1, :], float(BETA / D))
        wp2b = pers.tile([B, K], f32, name="wp2b")
        nc.gpsimd.partition_broadcast(wp2b, wp2r)
        xh_sb = pers.tile([B, K], f32, name="xh_sb")
        nc.vector.tensor_scalar_mul(xh_sb, ps_xh[:64, :], float(2.0 * BETA / D))
        nc.vector.tensor_sub(xh_sb, xh_sb, wp2b)

        def softmax(dst, src):
            nmax = scr.tile([B, 1], f32, name="smx_max")
            nc.vector.tensor_reduce(out=nmax, in_=src, axis=AX.X,
                                    op=ALU.max, negate=True)
            ssum = scr.tile([B, 1], f32, name="smx_sum")
            nc.scalar.activation(out=dst, in_=src, func=AF.Exp,
                                 bias=nmax, scale=1.0, accum_out=ssum)
            rcp = scr.tile([B, 1], f32, name="smx_rcp")
            nc.vector.reciprocal(rcp, ssum)
            nc.vector.tensor_scalar_mul(dst, dst, rcp)

        xp_sb = pers.tile([B, K], f32, name="xp_sb")
        softmax(xp_sb, xh_sb)

        # xpT (128, 4*64) f32 for lat1 lhsT
        ps_t = psm.tile([128, 512], f32, name="pscr")
        for q in range(4):
            nc.tensor.matmul(ps_t[:, q * 64:(q + 1) * 64],
                             lhsT=xp_sb[:, q * 128:(q + 1) * 128],
                             rhs=ident_sb[:64, :64], is_transpose=True,
                             start=(q == 0), stop=(q == 3))
        xpT_sb = pers.tile([128, 4 * 64], f32, name="xpT_sb")
        nc.vector.tensor_copy(xpT_sb, ps_t[:, :256])

        # xp_shard = xp @ Sel : own-core k-shard columns, SPMD-uniform indexing
        ps_sel = psm.tile([128, 512], f32, name="pscr")
        for q in range(4):
            nc.tensor.matmul(ps_sel[:64, :KC], lhsT=xpT_sb[:, q * 64:(q + 1) * 64],
                             rhs=sel_sb[:, q * KC:(q + 1) * KC],
                             start=(q == 0), stop=(q == 3))
        xp_shard = pers.tile([B, KC], f32, name="xp_shard")
        nc.vector.tensor_copy(xp_shard, ps_sel[:64, :KC])
        ps_xst = psm.tile([128, 512], f32, name="pscr")
        nc.tensor.matmul(ps_xst[:64, :64], lhsT=xp_shard, rhs=ident_sb[:64, :64],
                         is_transpose=True, start=True, stop=True)
        xp_shT = pers.tile([KC, B], f32, name="xp_shT")
        nc.vector.tensor_copy(xp_shT, ps_xst[:64, :64])

        # lat1 = xp @ wrec
        ps_l1 = psm.tile([128, 512], f32, name="pscr")
        for q in range(4):
            nc.tensor.matmul(ps_l1[:64, :64], lhsT=xpT_sb[:, q * 64:(q + 1) * 64],
                             rhs=wrec_t_sb[:, q * 64:(q + 1) * 64],
                             start=(q == 0), stop=(q == 3))
        lat1_sb = pers.tile([B, E2], f32, name="lat1_sb")
        nc.vector.tensor_copy(lat1_sb, ps_l1[:64, :64])

        ps_l1t = psm.tile([128, 512], f32, name="pscr")
        nc.tensor.matmul(ps_l1t[:64, :64], lhsT=lat1_sb, rhs=ident_sb[:64, :64],
                         is_transpose=True, start=True, stop=True)
        lat1T_sb = pers.tile([E2, B], f32, name="lat1T_sb")
        nc.vector.tensor_copy(lat1T_sb, ps_l1t[:64, :64])

        # xh2 logits = (2b/E2) lat1@w_rec - (b/E2)||wrec_k||^2
        ps_xh2 = psm.tile([128, 512], f32, name="pscr")
        nc.tensor.matmul(ps_xh2[:64, :], lhsT=lat1T_sb, rhs=wrec_n_sb,
                         start=True, stop=True)
        sq64 = scr.tile([E2, K], bf16, name="sq64")
        nc.scalar.activation(out=sq64, in_=wrec_n_sb, func=AF.Square)
        ps_wr2 = psm.tile([128, 512], f32, name="pscr")
        nc.tensor.matmul(ps_wr2[:1, :], lhsT=ones_bf[:64], rhs=sq64,
                         start=True, stop=True)
        wr2r = pers.tile([1, K], f32, name="wr2r")
        nc.vector.tensor_scalar_mul(wr2r, ps_wr2[:1, :], float(BETA / E2))
        wr2b = pers.tile([B, K], f32, name="wr2b")
        nc.gpsimd.partition_broadcast(wr2b, wr2r)
        xh2_sb = pers.tile([B, K], f32, name="xh2_sb")
        nc.vector.tensor_scalar_mul(xh2_sb, ps_xh2[:64, :], float(2.0 * BETA / E2))
        nc.vector.tensor_sub(xh2_sb, xh2_sb, wr2b)
        xp2_sb = pers.tile([B, K], f32, name="xp2_sb")
        softmax(xp2_sb, xh2_sb)

        # xp2T f16 (tail lhsT)
        ps_t2 = psm.tile([128, 512], f32, name="pscr")
        for q in range(4):
            nc.tensor.matmul(ps_t2[:, q * 64:(q + 1) * 64],
                             lhsT=xp2_sb[:, q * 128:(q + 1) * 128],
                             rhs=ident_sb[:64, :64], is_transpose=True,
                             start=(q == 0), stop=(q == 3))
        xp2T16 = pers.tile([128, 4 * 64], f16, name="xp2T16")
        nc.vector.tensor_copy(xp2T16, ps_t2[:, :256])

        # xxt base: xxt[p, cc*64+b] = xp2T[cc*2 + p//64, b]  (via DRAM round-trip)
        nc.gpsimd.dma_start(
            out=bass.AP(tensor=xp2_dr, offset=0, ap=[[64, 128], [64 * 128, 4], [1, B]]),
            in_=xp2T16.rearrange("p (q b) -> p q b", b=B))
        xxt = pers.tile([128, CC * B], f16, name="xxt")
        for p1 in range(2):
            dst = xxt[p1 * 64:(p1 + 1) * 64, :].rearrange("p (c b) -> p c b", b=B)
            src = bass.AP(tensor=xp2_dr, offset=p1 * B,
                          ap=[[0, 64], [2 * B, CC], [1, B]])
            nc.sync.dma_start(out=dst, in_=src)

        # ---- phase 1: encode over own k-shard ----
        s2d = pers.tile([KC, E2], f32, name="s2d")
        c2d = pers.tile([KC, E2], f32, name="c2d")
        lat2acc = pers.tile([B, E2], f32, name="lat2acc")
        nc.vector.memset(lat2acc, 0.0)

        for g in range(G):
            est = estp.tile([128, T * MSTAT], f16, name="est")
            nc.gpsimd.dma_start(out=est, in_=enc_st[:, g * T * MSTAT:(g + 1) * T * MSTAT])
            ew = ewp.tile([128, (T // 2) * J * E2], f16, name="ew")
            ew2 = ewp.tile([128, (T // 2) * J * E2], f16, name="ew")
            half = (T // 2) * J * E2
            nc.sync.dma_start(out=ew, in_=enc_w[:, g * T * J * E2:g * T * J * E2 + half])
            nc.scalar.dma_start(out=ew2, in_=enc_w[:, g * T * J * E2 + half:(g + 1) * T * J * E2])
            psA = psa.tile([128, 512], f32, name="psA")
            psS = pss.tile([1, 512], f32, name="psS")
            for t in range(T):
                buf = ew if t < T // 2 else ew2
                tt = t % (T // 2)
                nc.tensor.matmul(psA[:MSTAT, :],
                                 lhsT=est[:, t * MSTAT:(t + 1) * MSTAT],
                                 rhs=buf[:, tt * 512:(tt + 1) * 512],
                                 start=(t == 0), stop=(t == T - 1))
            for t in range(T):
                buf = ew if t < T // 2 else ew2
                tt = t % (T // 2)
                sq = sqp.tile([128, 512], bf16, name="sq")
                nc.scalar.activation(out=sq, in_=buf[:, tt * 512:(tt + 1) * 512],
                                     func=AF.Square)
                nc.tensor.matmul(psS[:1, :], lhsT=ones_bf, rhs=sq,
                                 start=(t == 0), stop=(t == T - 1))

            cs_g = scr.tile([1, 512], f32, name="cs_g")
            nc.vector.tensor_copy(cs_g, psA[64:65, :])
            cc32 = scr.tile([32, 512], f32, name="cc32")
            nc.vector.tensor_copy(cc32, psA[64:96, :])
            for j in range(J):
                # diagonal extraction: engines need aligned partition starts,
                # DMA does not
                nc.gpsimd.dma_start(out=c2d[g * J + j:g * J + j + 1, :],
                                    in_=cc32[1 + j:2 + j, j * 64:(j + 1) * 64])
            # var = ss/(D-1) - cs^2/(D(D-1)); s = 1/(0.01+sqrt(var))
            v1 = scr.tile([1, 512], f32, name="v1")
            v2 = scr.tile([1, 512], f32, name="v2")
            s_g = scr.tile([1, 512], f32, name="s_g")
            nc.vector.tensor_mul(v1, cs_g, cs_g)
            nc.vector.tensor_scalar_mul(v1, v1, float(1.0 / (D * (D - 1.0))))
            nc.vector.tensor_scalar_mul(v2, psS[:1, :], float(1.0 / (D - 1.0)))
            nc.vector.tensor_sub(v1, v2, v1)
            nc.scalar.activation(out=v1, in_=v1, func=AF.Sqrt)
            nc.vector.tensor_scalar_add(v1, v1, 0.01)
            nc.vector.reciprocal(s_g, v1)
            # persist s shard rows: s2d[g*J+j, e] = s_g[0, j*64+e]
            nc.gpsimd.dma_start(
                out=s2d[g * J:(g + 1) * J, :],
                in_=bass.AP(tensor=s_g.tensor, offset=s_g.offset,
                            ap=[[1, 1], [E2, J], [1, E2]]))
            # lat2 contribution: sum_j xp[:,shard g,j]*s*A  (c-term subtracted
            # once after the loop)
            sb_g = scr.tile([B, 512], f32, name="sb_g")
            nc.gpsimd.partition_broadcast(sb_g, s_g)
            a1 = scr.tile([B, 512], f32, name="a1")
            nc.vector.tensor_mul(a1, psA[:64, :], sb_g)
            xpj = xp_shard[:, g * J:(g + 1) * J]
            nc.vector.tensor_tensor(out=a1, in0=a1, in1=eb(xpj, E2), op=ALU.mult)
            red = scr.tile([B, E2], f32, name="red")
            nc.vector.tensor_reduce(out=red, in_=a1.rearrange("p (j e) -> p e j", j=J),
                                    axis=AX.X, op=ALU.add)
            nc.vector.tensor_add(lat2acc, lat2acc, red)

        # c-term: lat2acc -= xp_shard @ (c2d * s2d)
        csd = pers.tile([KC, E2], f32, name="csd")
        nc.vector.tensor_mul(csd, c2d, s2d)
        ps_c = psm.tile([128, 512], f32, name="pscr")
        nc.tensor.matmul(ps_c[:64, :64], lhsT=xp_shT, rhs=csd, start=True, stop=True)
        nc.vector.tensor_sub(lat2acc, lat2acc, ps_c[:64, :64])

        # ---- one AllGather: [s shard | lat2 partial] per rank ----
        nc.gpsimd.dma_start(out=bass.AP(tensor=sl_in, offset=0, ap=[[64, KC], [1, E2]]),
                            in_=s2d)
        nc.gpsimd.dma_start(out=bass.AP(tensor=sl_in, offset=SL, ap=[[64, B], [1, E2]]),
                            in_=lat2acc)
        nc.gpsimd.collective_compute("AllGather", ALU.bypass, replica_groups=RG,
                                     ins=[sl_in[:, :]], outs=[sl_ag[:, :]])
        # s_col[p, cc] = s_global[cc*128+p]; global ke spans rank rows
        s_col = pers.tile([128, CC], f32, name="s_col")
        CCR = CC // NCORES
        for r in range(NCORES):
            nc.gpsimd.dma_start(
                out=s_col[:, r * CCR:(r + 1) * CCR],
                in_=bass.AP(tensor=sl_ag, offset=r * 2 * SL,
                            ap=[[1, 128], [128, CCR]]))
        # xxt *= s  (s_col broadcast over b)
        s_b = bass.AP(tensor=s_col.tensor, offset=s_col.offset,
                      ap=[list(s_col.ap[0]), [1, CC], [0, B]])
        nc.vector.tensor_tensor(out=xxt.rearrange("p (c b) -> p c b", b=B),
                                in0=xxt.rearrange("p (c b) -> p c b", b=B),
                                in1=s_b, op=ALU.mult)

        # lat2 = sum over ranks of partials, replicated into (128, B):
        # lat2rep[p, b] = lat2[b, p%64] -- 16 accumulating SWDGE DMAs
        lat2rep = pers.tile([128, B], f32, name="lat2rep")
        for r in range(NCORES):
            for p1 in range(2):
                nc.gpsimd.dma_start(
                    out=lat2rep[p1 * 64:(p1 + 1) * 64, :],
                    in_=bass.AP(tensor=sl_ag, offset=r * 2 * SL + SL,
                                ap=[[1, 64], [64, B]]),
                    accum_op=(ALU.bypass if r == 0 else ALU.add))
        l_b = bass.AP(tensor=lat2rep.tensor, offset=lat2rep.offset,
                      ap=[list(lat2rep.ap[0]), [0, CC], [1, B]])
        nc.vector.tensor_tensor(out=xxt.rearrange("p (c b) -> p c b", b=B),
                                in0=xxt.rearrange("p (c b) -> p c b", b=B),
                                in1=l_b, op=ALU.mult)

        # ---- phase 2: decode over own d-slice ----
        ps_xd = psx.tile([B, DC], f32, name="ps_xd")
        for m in range(CC // DECSL):
            dw = dwp.tile([128, DECSL * DC], f16, name="dw")
            dmae = nc.sync if m % 2 == 0 else nc.scalar
            dmae.dma_start(out=dw, in_=dec_w[:, m * (DECSL * DC):(m + 1) * (DECSL * DC)])
            for i in range(DECSL):
                cc = m * DECSL + i
                nc.tensor.matmul(ps_xd, lhsT=xxt[:, cc * B:(cc + 1) * B],
                                 rhs=dw[:, i * DC:(i + 1) * DC],
                                 start=(cc == 0), stop=False)
        for q in range(4):
            nc.tensor.matmul(ps_xd, lhsT=xp2T16[:, q * 64:(q + 1) * 64],
                             rhs=wp_dec_sb[:, q * DC:(q + 1) * DC],
                             start=False, stop=(q == 3))

        if debug:
            xd2dbg = pers.tile([B, DC], f32, name="xd2dbg")
            nc.vector.tensor_copy(xd2dbg, ps_xd)
            nc.sync.dma_start(out=dbg["dbg_xd2"][:, :], in_=xd2dbg)

        rsub = pers.tile([B, DC], f32, name="rsub")
        nc.vector.tensor_sub(rsub, ps_xd, xdsl_sb)
        sqout = pers.tile([B, DC], f32, name="sqout")
        loss_col = pers.tile([B, 1], f32, name="loss_col")
        nc.scalar.activation(out=sqout, in_=rsub, func=AF.Square,
                             scale=float(1.0 / np.sqrt(D)), accum_out=loss_col)
        nc.gpsimd.dma_start(out=loss_in[:, :], in_=loss_col)
        nc.gpsimd.collective_compute("AllGather", ALU.bypass, replica_groups=RG,
                                     ins=[loss_in[:, :]], outs=[loss_ag[:, :]])
        lparts = pers.tile([B, NCORES], f32, name="lparts")
        nc.gpsimd.dma_start(out=lparts,
                            in_=bass.AP(tensor=loss_ag, offset=0,
                                        ap=[[1, B], [B, NCORES]]))
        lsum = pers.tile([B, 1], f32, name="lsum")
        nc.vector.tensor_reduce(out=lsum, in_=lparts, axis=AX.X, op=ALU.add)
        nc.sync.dma_start(out=loss_o[:], in_=lsum)

        if debug:
            nc.sync.dma_start(out=dbg["dbg_xp"][:, :], in_=xp_sb)
            nc.sync.dma_start(out=dbg["dbg_lat1"][:, :], in_=lat1_sb)
            nc.sync.dma_start(out=dbg["dbg_xp2"][:, :], in_=xp2_sb)
            nc.sync.dma_start(out=dbg["dbg_s"][:, :], in_=s2d)
            nc.sync.dma_start(out=dbg["dbg_lat2p"][:, :], in_=lat2acc)
            nc.sync.dma_start(out=bass.AP(tensor=dbg["dbg_lat2"], offset=0,
                                          ap=[[1, E2], [E2, B]]),
                              in_=lat2rep[:64, :])
            nc.sync.dma_start(out=dbg["dbg_xxt"][:, :], in_=xxt)

    nc.finalize()
    return nc


def pack_inputs(images, w_proj, w_img2, w_rec):
    """Shard + lay out the full inputs for the 8 cores (pure data movement
    plus fp16 casts; no arithmetic)."""
    images = np.ascontiguousarray(images, dtype=np.float32)
    w_proj = np.ascontiguousarray(w_proj, dtype=np.float32)
    w_img2 = np.ascontiguousarray(w_img2, dtype=np.float32)
    w_rec = np.ascontiguousarray(w_rec, dtype=np.float32)

    w3 = w_img2.reshape(K, D, E2)
    imT3 = images.T.reshape(T, 128, B)                     # [t, p, b]
    ident = np.eye(128, dtype=np.float32)

    # replicated tensors
    imT = np.ascontiguousarray(imT3.transpose(1, 0, 2).reshape(128, T * B)).astype(np.float16)
    wpT = np.ascontiguousarray(
        w_proj.T.reshape(T, 128, K).transpose(1, 0, 2).reshape(128, T * K)
    ).astype(np.float16)
    wrec_t = np.ascontiguousarray(
        w_rec.T.reshape(4, 128, E2).transpose(1, 0, 2).reshape(128, 4 * E2)
    ).astype(np.float32)
    wrec_n = w_rec.astype(np.float32)

    # decode layout: (k,e)-major, d-minor
    wdec_full = np.ascontiguousarray(w3.transpose(0, 2, 1)).reshape(K * E2, D)

    in_maps = []
    for c in range(NCORES):
        ks = slice(c * KC, (c + 1) * KC)
        dsl = slice(c * DC, (c + 1) * DC)

        enc = w3[ks].reshape(G, J, T, 128, E2).transpose(3, 0, 2, 1, 4)
        enc_w = np.ascontiguousarray(enc).reshape(128, G * T * J * E2).astype(np.float16)

        stat = np.zeros((128, G, T, MSTAT), dtype=np.float16)
        stat[:, :, :, :64] = imT3.transpose(1, 0, 2)[:, None, :, :].astype(np.float16)
        stat[:, :, :, 64] = 1.0
        wp = w_proj[ks].reshape(G, J, T, 128)              # [g, j, t, p]
        stat[:, :, :, 65:] = wp.transpose(3, 0, 2, 1).astype(np.float16)
        enc_st = np.ascontiguousarray(stat.reshape(128, G * T * MSTAT))

        dec = wdec_full[:, dsl].reshape(CC, 128, DC).transpose(1, 0, 2)
        dec_w = np.ascontiguousarray(dec).reshape(128, CC * DC).astype(np.float16)

        wp_dec = np.ascontiguousarray(
            w_proj[:, dsl].reshape(4, 128, DC).transpose(1, 0, 2).reshape(128, 4 * DC)
        ).astype(np.float16)

        sel_np = np.zeros((128, 4, KC), dtype=np.float32)
        for kk in range(KC):
            k = c * KC + kk
            sel_np[k % 128, k // 128, kk] = 1.0
        sel_np = sel_np.reshape(128, 4 * KC)

        in_maps.append({
            "enc_w": enc_w,
            "enc_st": enc_st,
            "imT": imT,
            "wpT": wpT,
            "dec_w": dec_w,
            "wp_dec": wp_dec,
            "wrec_t": wrec_t,
            "wrec_n": wrec_n,
            "x_dsl": np.ascontiguousarray(images[:, dsl]),
            "sel": sel_np,
            "ident": ident,
        })
    return in_maps


def run(images, w_proj, w_img2, w_rec, debug=False, trace=False):
    from concourse.bass_utils import run_bass_kernel_spmd

    key = bool(debug)
    if key not in _PROGRAM_CACHE:
        _PROGRAM_CACHE[key] = _build_program(debug=debug)
    nc = _PROGRAM_CACHE[key]
    in_maps = pack_inputs(images, w_proj, w_img2, w_rec)
    res = run_bass_kernel_spmd(nc, in_maps, list(range(NCORES)), trace=trace)
    return res


def kernel(images, w_proj, w_img2, w_rec):
    res = run(images, w_proj, w_img2, w_rec)
    return np.asarray(res.results[0]["loss"], dtype=np.float32)


# revision 38
# speedup vs baseline: 1.4384x; 1.4384x over previous
"""Trainium2 Bass kernel for nn_MixedDiffEncoder (vq_codebook).

Math (see reference):
  loss[b] = mean_d( (xp2 @ w_proj + xd2 - images)^2 )
with the two heavy contractions against the (K, D*E2) codebook w_img2
restructured into dense matmuls:
  encode:  A_raw[b,k,e] = sum_d images[b,d] * w_img2[k,d,e]   (k-sharded)
           lat2[b,e]    = sum_k xp[b,k]*s[k,e]*(A_raw - c_raw)[b,k,e]
  decode:  xd2[b,d]     = sum_{k,e} xp2[b,k]*lat2[b,e]*s[k,e]*w_img2[k,d,e]
           (d-sharded, host-pretransposed to (k,e)-major layout)
s[k,e] = 1/(0.01 + std_d) is computed on-device from column sums (a ones
column folded into the encode stationary) and square-sums (scalar-engine
square + ones-matmul). Softmaxes drop per-row constants (invariant).

Sharding over 8 cores: encode k-parallel (64 k each), decode d-parallel
(384 d each). Cross-core collectives: AllGather of s shards, AllReduce of
lat2 partials, AllReduce of per-d-slice loss partials.
"""

import sys

for _p in ("/opt/trn_rl_repo", "/root/.axon_site/_ro/trn_rl_repo"):
    if _p not in sys.path:
        sys.path.append(_p)

import numpy as np

B, D, K, E2 = 64, 3072, 512, 64
NCORES = 8
KC = K // NCORES          # 64 k per core (encode shard)
DC = D // NCORES          # 384 d per core (decode shard)
G = 8                     # k-groups per core
J = KC // G               # 8 k per group
T = D // 128              # 24 contraction chunks
MSTAT = 64 + 1 + J        # 73 stationary columns (imagesT | ones | wprojT)
CC = (K * E2) // 128      # 256 decode contraction chunks
DECSL = 16                # decode chunks per DMA slice
WPQ = 6                   # wpT chunks per DMA slice
BETA = 5.0

_PROGRAM_CACHE = {}


def _build_program(debug=False):
    import concourse.bass as bass
    import concourse.bacc as bacc
    import concourse.mybir as mybir
    import concourse.tile as tile
    from contextlib import ExitStack

    f16 = mybir.dt.float16
    f32 = mybir.dt.float32
    bf16 = mybir.dt.bfloat16
    AF = mybir.ActivationFunctionType
    ALU = mybir.AluOpType
    AX = mybir.AxisListType
    RG = [list(range(NCORES))]

    nc = bacc.Bacc()

    # ---- I/O ----
    enc_w = nc.declare_dram_parameter("enc_w", [128, G * T * J * E2], f16, isOutput=False)
    enc_st = nc.declare_dram_parameter("enc_st", [128, G * T * MSTAT], f16, isOutput=False)
    imT = nc.declare_dram_parameter("imT", [128, T * 64], f16, isOutput=False)
    wpT = nc.declare_dram_parameter("wpT", [128, T * K], f16, isOutput=False)
    dec_w = nc.declare_dram_parameter("dec_w", [128, CC * DC], f16, isOutput=False)
    wp_dec = nc.declare_dram_parameter("wp_dec", [128, 4 * DC], f16, isOutput=False)
    wrec_t = nc.declare_dram_parameter("wrec_t", [128, 4 * E2], f32, isOutput=False)
    wrec_n = nc.declare_dram_parameter("wrec_n", [E2, K], f32, isOutput=False)
    x_dsl = nc.declare_dram_parameter("x_dsl", [B, DC], f32, isOutput=False)
    sel = nc.declare_dram_parameter("sel", [128, 4 * KC], f32, isOutput=False)
    ident = nc.declare_dram_parameter("ident", [128, 128], f32, isOutput=False)
    loss_o = nc.declare_dram_parameter("loss", [B], f32, isOutput=True)

    dbg = {}
    if debug:
        for name, shape in [
            ("dbg_xp", [B, K]), ("dbg_lat1", [B, E2]), ("dbg_xp2", [B, K]),
            ("dbg_s", [KC, E2]), ("dbg_lat2p", [B, E2]),
            ("dbg_lat2", [B, E2]), ("dbg_xd2", [B, DC]),
        ]:
            dbg[name] = nc.declare_dram_parameter(name, shape, f32, isOutput=True)
        dbg["dbg_xxt"] = nc.declare_dram_parameter("dbg_xxt", [128, CC * B], f16,
                                                   isOutput=True)

    # ---- internal DRAM (collective bounce buffers) ----
    # one AllGather carries both the s shard (first 4096) and the lat2
    # partial (last 4096) per rank
    SL = KC * E2              # 4096
    sl_in = nc.dram_tensor("sl_in", [1, 2 * SL], f32)
    sl_ag = nc.dram_tensor("sl_ag", [NCORES, 2 * SL], f32, addr_space="Shared")
    loss_in = nc.dram_tensor("loss_in", [B, 1], f32)
    loss_ag = nc.dram_tensor("loss_ag", [NCORES * B, 1], f32, addr_space="Shared")

    def pb(ap, n):
        """Broadcast a (1, ...) AP across n partitions (partition step 0)."""
        return bass.AP(tensor=ap.tensor, offset=ap.offset,
                       ap=[[0, n]] + [list(d) for d in ap.ap[1:]])

    def eb(ap, n):
        """Append an innermost free broadcast dim (step 0, count n)."""
        return bass.AP(tensor=ap.tensor, offset=ap.offset,
                       ap=[list(d) for d in ap.ap] + [[0, n]])

    with tile.TileContext(nc) as tc, ExitStack() as ctx:
        const = ctx.enter_context(tc.tile_pool(name="const", bufs=1))
        pers = ctx.enter_context(tc.tile_pool(name="pers", bufs=1))
        scr = ctx.enter_context(tc.tile_pool(name="scr", bufs=2))
        estp = ctx.enter_context(tc.tile_pool(name="estp", bufs=2))
        wptp = ctx.enter_context(tc.tile_pool(name="wptp", bufs=2))
        ewp = ctx.enter_context(tc.tile_pool(name="ewp", bufs=2))
        dwp = ctx.enter_context(tc.tile_pool(name="dwp", bufs=2))
        sqp = ctx.enter_context(tc.tile_pool(name="sqp", bufs=3))
        psm = ctx.enter_context(tc.tile_pool(name="psm", bufs=3, space="PSUM"))
        psa = ctx.enter_context(tc.tile_pool(name="psa", bufs=2, space="PSUM"))
        pss = ctx.enter_context(tc.tile_pool(name="pss", bufs=2, space="PSUM"))
        psx = ctx.enter_context(tc.tile_pool(name="psx", bufs=1, space="PSUM"))

        # ---- constant loads ----
        ident_sb = const.tile([128, 128], f32, name="ident_sb")
        nc.sync.dma_start(out=ident_sb, in_=ident[:, :])
        ident16 = const.tile([64, 64], f16, name="ident16")
        nc.vector.tensor_copy(ident16, ident_sb[:64, :64])
        ones_bf = const.tile([128, 1], bf16, name="ones_bf")
        nc.vector.memset(ones_bf, 1.0)
        imT_sb = const.tile([128, T * 64], f16, name="imT_sb")
        nc.sync.dma_start(out=imT_sb, in_=imT[:, :])
        wrec_t_sb = const.tile([128, 4 * E2], f32, name="wrec_t_sb")
        nc.sync.dma_start(out=wrec_t_sb, in_=wrec_t[:, :])
        wrec_n_sb = const.tile([E2, K], f32, name="wrec_n_sb")
        nc.sync.dma_start(out=wrec_n_sb, in_=wrec_n[:, :])
        xdsl_sb = const.tile([B, DC], f32, name="xdsl_sb")
        nc.sync.dma_start(out=xdsl_sb, in_=x_dsl[:, :])
        sel_sb = const.tile([128, 4 * KC], f32, name="sel_sb")
        nc.sync.dma_start(out=sel_sb, in_=sel[:, :])
        wp_dec_sb = const.tile([128, 4 * DC], f16, name="wp_dec_sb")
        nc.sync.dma_start(out=wp_dec_sb, in_=wp_dec[:, :])

        # ---- phase 0: xh logits = (2b/D) images@w_proj.T - (b/D)||w_k||^2 ----
        ps_xh = psm.tile([128, 512], f32, name="pscr")
        ps_wp2 = psm.tile([128, 512], f32, name="pscr")
        for q in range(T // WPQ):
            wpq = wptp.tile([128, WPQ * K], f16, name="wpq")
            nc.sync.dma_start(out=wpq, in_=wpT[:, q * WPQ * K:(q + 1) * WPQ * K])
            for i in range(WPQ):
                t = q * WPQ + i
                nc.tensor.matmul(ps_xh[:64, :], lhsT=imT_sb[:, t * 64:(t + 1) * 64],
                                 rhs=wpq[:, i * K:(i + 1) * K],
                                 start=(t == 0), stop=(t == T - 1))
            for i in range(WPQ):
                t = q * WPQ + i
                sq = sqp.tile([128, 512], bf16, name="sq")
                nc.scalar.activation(out=sq, in_=wpq[:, i * K:(i + 1) * K], func=AF.Square)
                nc.tensor.matmul(ps_wp2[:1, :], lhsT=ones_bf, rhs=sq,
                                 start=(t == 0), stop=(t == T - 1))

        wp2r = pers.tile([1, K], f32, name="wp2r")
        nc.vector.tensor_scalar_mul(wp2r, ps_wp2[:1, :], float(BETA / D))
        wp2b = pers.tile([B, K], f32, name="wp2b")
        nc.gpsimd.partition_broadcast(wp2b, wp2r)
        xh_sb = pers.tile([B, K], f32, name="xh_sb")
        nc.vector.tensor_scalar_mul(xh_sb, ps_xh[:64, :], float(2.0 * BETA / D))
        nc.vector.tensor_sub(xh_sb, xh_sb, wp2b)

        def softmax(dst, src):
            nmax = scr.tile([B, 1], f32, name="smx_max")
            nc.vector.tensor_reduce(out=nmax, in_=src, axis=AX.X,
                                    op=ALU.max, negate=True)
            ssum = scr.tile([B, 1], f32, name="smx_sum")
            nc.scalar.activation(out=dst, in_=src, func=AF.Exp,
                                 bias=nmax, scale=1.0, accum_out=ssum)
            rcp = scr.tile([B, 1], f32, name="smx_rcp")
            nc.vector.reciprocal(rcp, ssum)
            nc.vector.tensor_scalar_mul(dst, dst, rcp)

        xp_sb = pers.tile([B, K], f32, name="xp_sb")
        softmax(xp_sb, xh_sb)

        # xpT (128, 4*64) f32 for lat1 lhsT
        ps_t = psm.tile([128, 512], f32, name="pscr")
        for q in range(4):
            nc.tensor.matmul(ps_t[:, q * 64:(q + 1) * 64],
                             lhsT=xp_sb[:, q * 128:(q + 1) * 128],
                             rhs=ident_sb[:64, :64], is_transpose=True,
                             start=(q == 0), stop=(q == 3))
        xpT_sb = pers.tile([128, 4 * 64], f32, name="xpT_sb")
        nc.vector.tensor_copy(xpT_sb, ps_t[:, :256])

        # xp_shard = xp @ Sel : own-core k-shard columns, SPMD-uniform indexing
        ps_sel = psm.tile([128, 512], f32, name="pscr")
        for q in range(4):
            nc.tensor.matmul(ps_sel[:64, :KC], lhsT=xpT_sb[:, q * 64:(q + 1) * 64],
                             rhs=sel_sb[:, q * KC:(q + 1) * KC],
                             start=(q == 0), stop=(q == 3))
        xp_shard = pers.tile([B, KC], f32, name="xp_shard")
        nc.vector.tensor_copy(xp_shard, ps_sel[:64, :KC])
        ps_xst = psm.tile([128, 512], f32, name="pscr")
        nc.tensor.matmul(ps_xst[:64, :64], lhsT=xp_shard, rhs=ident_sb[:64, :64],
                         is_transpose=True, start=True, stop=True)
        xp_shT = pers.tile([KC, B], f32, name="xp_shT")
        nc.vector.tensor_copy(xp_shT, ps_xst[:64, :64])

        # lat1 = xp @ wrec
        ps_l1 = psm.tile([128, 512], f32, name="pscr")
        for q in range(4):
            nc.tensor.matmul(ps_l1[:64, :64], lhsT=xpT_sb[:, q * 64:(q + 1) * 64],
                             rhs=wrec_t_sb[:, q * 64:(q + 1) * 64],
                             start=(q == 0), stop=(q == 3))
        lat1_sb = pers.tile([B, E2], f32, name="lat1_sb")
        nc.vector.tensor_copy(lat1_sb, ps_l1[:64, :64])

        ps_l1t = psm.tile([128, 512], f32, name="pscr")
        nc.tensor.matmul(ps_l1t[:64, :64], lhsT=lat1_sb, rhs=ident_sb[:64, :64],
                         is_transpose=True, start=True, stop=True)
        lat1T_sb = pers.tile([E2, B], f32, name="lat1T_sb")
        nc.vector.tensor_copy(lat1T_sb, ps_l1t[:64, :64])

        # xh2 logits = (2b/E2) lat1@w_rec - (b/E2)||wrec_k||^2
        ps_xh2 = psm.tile([128, 512], f32, name="pscr")
        nc.tensor.matmul(ps_xh2[:64, :], lhsT=lat1T_sb, rhs=wrec_n_sb,
                         start=True, stop=True)
        sq64 = scr.tile([E2, K], bf16, name="sq64")
        nc.scalar.activation(out=sq64, in_=wrec_n_sb, func=AF.Square)
        ps_wr2 = psm.tile([128, 512], f32, name="pscr")
        nc.tensor.matmul(ps_wr2[:1, :], lhsT=ones_bf[:64], rhs=sq64,
                         start=True, stop=True)
        wr2r = pers.tile([1, K], f32, name="wr2r")
        nc.vector.tensor_scalar_mul(wr2r, ps_wr2[:1, :], float(BETA / E2))
        wr2b = pers.tile([B, K], f32, name="wr2b")
        nc.gpsimd.partition_broadcast(wr2b, wr2r)
        xh2_sb = pers.tile([B, K], f32, name="xh2_sb")
        nc.vector.tensor_scalar_mul(xh2_sb, ps_xh2[:64, :], float(2.0 * BETA / E2))
        nc.vector.tensor_sub(xh2_sb, xh2_sb, wr2b)
        xp2_sb = pers.tile([B, K], f32, name="xp2_sb")
        softmax(xp2_sb, xh2_sb)

        # xp2T f16 (tail lhsT)
        ps_t2 = psm.tile([128, 512], f32, name="pscr")
        for q in range(4):
            nc.tensor.matmul(ps_t2[:, q * 64:(q + 1) * 64],
                             lhsT=xp2_sb[:, q * 128:(q + 1) * 128],
                             rhs=ident_sb[:64, :64], is_transpose=True,
                             start=(q == 0), stop=(q == 3))
        xp2T16 = pers.tile([128, 4 * 64], f16, name="xp2T16")
        nc.vector.tensor_copy(xp2T16, ps_t2[:, :256])

        xxt = pers.tile([128, CC * B], f16, name="xxt")

        # ---- phase 1: encode over own k-shard ----
        s2d = pers.tile([KC, E2], f32, name="s2d")
        c2d = pers.tile([KC, E2], f32, name="c2d")
        lat2acc = pers.tile([B, E2], f32, name="lat2acc")
        nc.vector.memset(lat2acc, 0.0)

        for g in range(G):
            est = estp.tile([128, T * MSTAT], f16, name="est")
            nc.gpsimd.dma_start(out=est, in_=enc_st[:, g * T * MSTAT:(g + 1) * T * MSTAT])
            ew = ewp.tile([128, (T // 2) * J * E2], f16, name="ew")
            ew2 = ewp.tile([128, (T // 2) * J * E2], f16, name="ew")
            half = (T // 2) * J * E2
            nc.sync.dma_start(out=ew, in_=enc_w[:, g * T * J * E2:g * T * J * E2 + half])
            nc.sync.dma_start(out=ew2, in_=enc_w[:, g * T * J * E2 + half:(g + 1) * T * J * E2])
            psA = psa.tile([128, 512], f32, name="psA")
            psS = pss.tile([1, 512], f32, name="psS")
            for t in range(T):
                buf = ew if t < T // 2 else ew2
                tt = t % (T // 2)
                nc.tensor.matmul(psA[:MSTAT, :],
                                 lhsT=est[:, t * MSTAT:(t + 1) * MSTAT],
                                 rhs=buf[:, tt * 512:(tt + 1) * 512],
                                 start=(t == 0), stop=(t == T - 1))
            for t in range(T):
                buf = ew if t < T // 2 else ew2
                tt = t % (T // 2)
                sq = sqp.tile([128, 512], bf16, name="sq")
                nc.scalar.activation(out=sq, in_=buf[:, tt * 512:(tt + 1) * 512],
                                     func=AF.Square)
                nc.tensor.matmul(psS[:1, :], lhsT=ones_bf, rhs=sq,
                                 start=(t == 0), stop=(t == T - 1))

            cs_g = scr.tile([1, 512], f32, name="cs_g")
            nc.vector.tensor_copy(cs_g, psA[64:65, :])
            cc32 = scr.tile([32, 512], f32, name="cc32")
            nc.vector.tensor_copy(cc32, psA[64:96, :])
            for j in range(J):
                # diagonal extraction: engines need aligned partition starts,
                # DMA does not
                nc.gpsimd.dma_start(out=c2d[g * J + j:g * J + j + 1, :],
                                    in_=cc32[1 + j:2 + j, j * 64:(j + 1) * 64])
            # var = ss/(D-1) - cs^2/(D(D-1)); s = 1/(0.01+sqrt(var))
            v1 = scr.tile([1, 512], f32, name="v1")
            v2 = scr.tile([1, 512], f32, name="v2")
            s_g = scr.tile([1, 512], f32, name="s_g")
            nc.vector.tensor_mul(v1, cs_g, cs_g)
            nc.vector.tensor_scalar_mul(v1, v1, float(1.0 / (D * (D - 1.0))))
            nc.vector.tensor_scalar_mul(v2, psS[:1, :], float(1.0 / (D - 1.0)))
            nc.vector.tensor_sub(v1, v2, v1)
            nc.scalar.activation(out=v1, in_=v1, func=AF.Sqrt)
            nc.vector.tensor_scalar_add(v1, v1, 0.01)
            nc.vector.reciprocal(s_g, v1)
            # persist s shard rows: s2d[g*J+j, e] = s_g[0, j*64+e]
            nc.gpsimd.dma_start(
                out=s2d[g * J:(g + 1) * J, :],
                in_=bass.AP(tensor=s_g.tensor, offset=s_g.offset,
                            ap=[[1, 1], [E2, J], [1, E2]]))
            # lat2 contribution: sum_j xp[:,shard g,j]*s*A  (c-term subtracted
            # once after the loop)
            sb_g = scr.tile([B, 512], f32, name="sb_g")
            nc.gpsimd.partition_broadcast(sb_g, s_g)
            a1 = scr.tile([B, 512], f32, name="a1")
            nc.vector.tensor_mul(a1, psA[:64, :], sb_g)
            xpj = xp_shard[:, g * J:(g + 1) * J]
            nc.vector.tensor_tensor(out=a1, in0=a1, in1=eb(xpj, E2), op=ALU.mult)
            red = scr.tile([B, E2], f32, name="red")
            nc.vector.tensor_reduce(out=red, in_=a1.rearrange("p (j e) -> p e j", j=J),
                                    axis=AX.X, op=ALU.add)
            nc.vector.tensor_add(lat2acc, lat2acc, red)

        # c-term: lat2acc -= xp_shard @ (c2d * s2d)
        csd = pers.tile([KC, E2], f32, name="csd")
        nc.vector.tensor_mul(csd, c2d, s2d)
        ps_c = psm.tile([128, 512], f32, name="pscr")
        nc.tensor.matmul(ps_c[:64, :64], lhsT=xp_shT, rhs=csd, start=True, stop=True)
        nc.vector.tensor_sub(lat2acc, lat2acc, ps_c[:64, :64])

        # ---- one AllGather: [s shard | lat2 partial] per rank ----
        nc.gpsimd.dma_start(out=bass.AP(tensor=sl_in, offset=0, ap=[[64, KC], [1, E2]]),
                            in_=s2d)
        nc.gpsimd.dma_start(out=bass.AP(tensor=sl_in, offset=SL, ap=[[64, B], [1, E2]]),
                            in_=lat2acc)
        nc.gpsimd.collective_compute("AllGather", ALU.bypass, replica_groups=RG,
                                     ins=[sl_in[:, :]], outs=[sl_ag[:, :]])
        # s_col[p, cc] = s_global[cc*128+p]
        s_col = pers.tile([128, CC], f32, name="s_col")
        CCR = CC // NCORES
        for r in range(NCORES):
            nc.gpsimd.dma_start(
                out=s_col[:, r * CCR:(r + 1) * CCR],
                in_=bass.AP(tensor=sl_ag, offset=r * 2 * SL,
                            ap=[[1, 128], [128, CCR]]))
        # lat2 = sum over rank partials: one DMA gather + one DVE reduce
        lp8 = pers.tile([B, NCORES * E2], f32, name="lp8")
        nc.gpsimd.dma_start(out=lp8,
                            in_=bass.AP(tensor=sl_ag, offset=SL,
                                        ap=[[E2, B], [2 * SL, NCORES], [1, E2]]))
        lat2_sb = pers.tile([B, E2], f32, name="lat2_sb")
        nc.vector.tensor_reduce(out=lat2_sb,
                                in_=lp8.rearrange("p (r e) -> p e r", r=NCORES),
                                axis=AX.X, op=ALU.add)

        # xxt chunks: per block of 8 chunks, one DVE op forms
        # xx0[b,(k,e)] = xp2[b,k]*lat2[b,e] (both free-broadcast), 8 PE
        # transposes into one PSUM bank, then one DVE op that copies
        # PSUM->SBUF fused with the s multiply
        for blk in range(CC // 8):
            xx0b = scr.tile([B, 1024], f16, name="xx0b")
            in_xp2 = bass.AP(tensor=xp2_sb.tensor,
                             offset=xp2_sb.offset + blk * 16,
                             ap=[list(xp2_sb.ap[0]), [1, 16], [0, E2]])
            in_lat2 = bass.AP(tensor=lat2_sb.tensor, offset=lat2_sb.offset,
                              ap=[list(lat2_sb.ap[0]), [0, 16], [1, E2]])
            nc.vector.tensor_tensor(out=xx0b.rearrange("p (k e) -> p k e", k=16),
                                    in0=in_xp2, in1=in_lat2, op=ALU.mult)
            ps_tx = psm.tile([128, 512], f16, name="pscr")
            for i in range(8):
                nc.tensor.matmul(ps_tx[:, i * 64:(i + 1) * 64],
                                 lhsT=xx0b[:, i * 128:(i + 1) * 128],
                                 rhs=ident16, is_transpose=True,
                                 start=(i == 0), stop=(i == 7))
            s_slice = bass.AP(tensor=s_col.tensor,
                              offset=s_col.offset + blk * 8,
                              ap=[list(s_col.ap[0]), [1, 8], [0, B]])
            nc.vector.tensor_tensor(
                out=xxt[:, blk * 512:(blk + 1) * 512].rearrange(
                    "p (c b) -> p c b", b=B),
                in0=ps_tx.rearrange("p (c b) -> p c b", b=B),
                in1=s_slice, op=ALU.mult)

        # ---- phase 2: decode over own d-slice ----
        ps_xd = psx.tile([B, DC], f32, name="ps_xd")
        for m in range(CC // DECSL):
            dw = dwp.tile([128, DECSL * DC], f16, name="dw")
            dmae = nc.sync if m % 2 == 0 else nc.scalar
            dmae.dma_start(out=dw, in_=dec_w[:, m * (DECSL * DC):(m + 1) * (DECSL * DC)])
            for i in range(DECSL):
                cc = m * DECSL + i
                nc.tensor.matmul(ps_xd, lhsT=xxt[:, cc * B:(cc + 1) * B],
                                 rhs=dw[:, i * DC:(i + 1) * DC],
                                 start=(cc == 0), stop=False)
        for q in range(4):
            nc.tensor.matmul(ps_xd, lhsT=xp2T16[:, q * 64:(q + 1) * 64],
                             rhs=wp_dec_sb[:, q * DC:(q + 1) * DC],
                             start=False, stop=(q == 3))

        if debug:
            xd2dbg = pers.tile([B, DC], f32, name="xd2dbg")
            nc.vector.tensor_copy(xd2dbg, ps_xd)
            nc.sync.dma_start(out=dbg["dbg_xd2"][:, :], in_=xd2dbg)

        rsub = pers.tile([B, DC], f32, name="rsub")
        nc.vector.tensor_sub(rsub, ps_xd, xdsl_sb)
        sqout = pers.tile([B, DC], f32, name="sqout")
        loss_col = pers.tile([B, 1], f32, name="loss_col")
        nc.scalar.activation(out=sqout, in_=rsub, func=AF.Square,
                             scale=float(1.0 / np.sqrt(D)), accum_out=loss_col)
        nc.gpsimd.dma_start(out=loss_in[:, :], in_=loss_col)
        nc.gpsimd.collective_compute("AllGather", ALU.bypass, replica_groups=RG,
                                     ins=[loss_in[:, :]], outs=[loss_ag[:, :]])
        lparts = pers.tile([B, NCORES], f32, name="lparts")
        nc.gpsimd.dma_start(out=lparts,
                            in_=bass.AP(tensor=loss_ag, offset=0,
                                        ap=[[1, B], [B, NCORES]]))
        lsum = pers.tile([B, 1], f32, name="lsum")
        nc.vector.tensor_reduce(out=lsum, in_=lparts, axis=AX.X, op=ALU.add)
        nc.sync.dma_start(out=loss_o[:], in_=lsum)

        if debug:
            nc.sync.dma_start(out=dbg["dbg_xp"][:, :], in_=xp_sb)
            nc.sync.dma_start(out=dbg["dbg_lat1"][:, :], in_=lat1_sb)
            nc.sync.dma_start(out=dbg["dbg_xp2"][:, :], in_=xp2_sb)
            nc.sync.dma_start(out=dbg["dbg_s"][:, :], in_=s2d)
            nc.sync.dma_start(out=dbg["dbg_lat2p"][:, :], in_=lat2acc)
            nc.sync.dma_start(out=dbg["dbg_lat2"][:, :], in_=lat2_sb)
            nc.sync.dma_start(out=dbg["dbg_xxt"][:, :], in_=xxt)

    nc.finalize()
    return nc


def pack_inputs(images, w_proj, w_img2, w_rec):
    """Shard + lay out the full inputs for the 8 cores (pure data movement
    plus fp16 casts; no arithmetic)."""
    images = np.ascontiguousarray(images, dtype=np.float32)
    w_proj = np.ascontiguousarray(w_proj, dtype=np.float32)
    w_img2 = np.ascontiguousarray(w_img2, dtype=np.float32)
    w_rec = np.ascontiguousarray(w_rec, dtype=np.float32)

    w3 = w_img2.reshape(K, D, E2)
    imT3 = images.T.reshape(T, 128, B)                     # [t, p, b]
    ident = np.eye(128, dtype=np.float32)

    # replicated tensors
    imT = np.ascontiguousarray(imT3.transpose(1, 0, 2).reshape(128, T * B)).astype(np.float16)
    wpT = np.ascontiguousarray(
        w_proj.T.reshape(T, 128, K).transpose(1, 0, 2).reshape(128, T * K)
    ).astype(np.float16)
    wrec_t = np.ascontiguousarray(
        w_rec.T.reshape(4, 128, E2).transpose(1, 0, 2).reshape(128, 4 * E2)
    ).astype(np.float32)
    wrec_n = w_rec.astype(np.float32)

    # decode layout: (k,e)-major, d-minor
    wdec_full = np.ascontiguousarray(w3.transpose(0, 2, 1)).reshape(K * E2, D)

    in_maps = []
    for c in range(NCORES):
        ks = slice(c * KC, (c + 1) * KC)
        dsl = slice(c * DC, (c + 1) * DC)

        enc = w3[ks].reshape(G, J, T, 128, E2).transpose(3, 0, 2, 1, 4)
        enc_w = np.ascontiguousarray(enc).reshape(128, G * T * J * E2).astype(np.float16)

        stat = np.zeros((128, G, T, MSTAT), dtype=np.float16)
        stat[:, :, :, :64] = imT3.transpose(1, 0, 2)[:, None, :, :].astype(np.float16)
        stat[:, :, :, 64] = 1.0
        wp = w_proj[ks].reshape(G, J, T, 128)              # [g, j, t, p]
        stat[:, :, :, 65:] = wp.transpose(3, 0, 2, 1).astype(np.float16)
        enc_st = np.ascontiguousarray(stat.reshape(128, G * T * MSTAT))

        dec = wdec_full[:, dsl].reshape(CC, 128, DC).transpose(1, 0, 2)
        dec_w = np.ascontiguousarray(dec).reshape(128, CC * DC).astype(np.float16)

        wp_dec = np.ascontiguousarray(
            w_proj[:, dsl].reshape(4, 128, DC).transpose(1, 0, 2).reshape(128, 4 * DC)
        ).astype(np.float16)

        sel_np = np.zeros((128, 4, KC), dtype=np.float32)
        for kk in range(KC):
            k = c * KC + kk
            sel_np[k % 128, k // 128, kk] = 1.0
        sel_np = sel_np.reshape(128, 4 * KC)

        in_maps.append({
            "enc_w": enc_w,
            "enc_st": enc_st,
            "imT": imT,
            "wpT": wpT,
            "dec_w": dec_w,
            "wp_dec": wp_dec,
            "wrec_t": wrec_t,
            "wrec_n": wrec_n,
            "x_dsl": np.ascontiguousarray(images[:, dsl]),
            "sel": sel_np,
            "ident": ident,
        })
    return in_maps


def run(images, w_proj, w_img2, w_rec, debug=False, trace=False):
    from concourse.bass_utils import run_bass_kernel_spmd

    key = bool(debug)
    if key not in _PROGRAM_CACHE:
        _PROGRAM_CACHE[key] = _build_program(debug=debug)
    nc = _PROGRAM_CACHE[key]
    in_maps = pack_inputs(images, w_proj, w_img2, w_rec)
    res = run_bass_kernel_spmd(nc, in_maps, list(range(NCORES)), trace=trace)
    return res


def kernel(images, w_proj, w_img2, w_rec):
    res = run(images, w_proj, w_img2, w_rec)
    return np.asarray(res.results[0]["loss"], dtype=np.float32)
